# revision 18
# baseline (speedup 1.0000x reference)
"""Distributed Trainium2 kernel for the per-token-weight MLP head.

Math (T = B*L = 1024 tokens, N = H = 512, VOCAB = 32000):
    x     = relu(r @ W1.T + b1)                      (T, H)
    W_att = (x @ W2.T + b2).reshape(T, H, N)
    r_ol  = einsum('thn,tn->th', W_att, r)           (T, H)
    out   = r_ol @ W_out.T + b_out                   (T, VOCAB)

Key identity (fc2 output, 1 GB, is never materialized):
    r_ol[t,h] = sum_{n,k} W2[h*N+n, k] * x[t,k] * r[t,n]
              + sum_n    b2[h*N+n] * r[t,n]
For a fixed n this is a plain matmul over k of the row-scaled activations
Xn[k,t] = x[t,k]*r[t,n] against A_n[k,h] = W2[h*N+n, k].  All (n, k)
contributions accumulate into PSUM-resident r_ol^T tiles.

Sharding (8 cores): the n-dimension of the bmm contraction (64 values per
core, the matching slice of fc2's output dim) is tensor-parallel; partial
r_ol^T is AllReduced; the output projection is vocab-sharded (4096 rows
per core, vocab padded 32000->32768).  fc1 (tiny) is replicated.

Pipelining: work is split into 2 token-halves x 2 n-segments = 4 segments.
Each segment accumulates its partial r_ol^T in its own 4 PSUM banks; when
it finishes, the partial is AllReduced (bf16, 0.5 MB) while the next
segment computes.  Segment AR outputs are summed on-device (the reduction
is linear), so only the last segment's AR is ever exposed, and it overlaps
the first token-half's output projection.  Compute dtype bf16, PSUM f32.
"""

import sys

for _p in ("/opt/trn_rl_repo",):
    if _p not in sys.path:
        sys.path.append(_p)

import numpy as np
import ml_dtypes

BF16 = ml_dtypes.bfloat16

T = 1024          # B * L tokens
N = 512           # r feature dim (bmm contraction)
H = 512           # hidden dim == fc1 out == r_ol dim
K = 512           # fc1 output feature dim (= H) used as fc2 contraction
VOCAB = 32000
VPAD = 32768      # vocab padded to 8*4096
CORES = 8
NLOC = N // CORES          # 64 n-values per core
VLOC = VPAD // CORES       # 4096 vocab rows per core
P = 128                    # partitions
KT = K // P                # 4 k-chunks
HT = H // P                # 4 h-tiles
NCH = N // P               # 4 n-chunks (fc1 contraction)
TTH = 512                  # token-half width == matmul free dim
NTH = T // TTH             # 2 token halves
NSEG = 2                   # n-segments per token half
NSEGSZ = NLOC // NSEG      # 32 n per segment

_BUILT = None  # (nc) cache


def _build():
    import concourse.bass as bass
    import concourse.mybir as mybir
    import concourse.tile as tile
    from concourse import bacc

    from concourse.tile import add_dep_helper

    fp32 = mybir.dt.float32
    bf16 = mybir.dt.bfloat16
    AF = mybir.ActivationFunctionType

    nc = bacc.Bacc(
        "TRN2",
        target_bir_lowering=False,
        debug=False,
        num_devices=CORES,
    )

    # ---- I/O ----
    rT = nc.dram_tensor("rT", [N, T], bf16, kind="ExternalInput")        # replicated
    rloc = nc.dram_tensor("rloc", [NLOC, T], bf16, kind="ExternalInput")  # local n rows
    w1t = nc.dram_tensor("w1t", [N, K], bf16, kind="ExternalInput")      # W1.T (n,k)
    b1c = nc.dram_tensor("b1c", [P, KT], fp32, kind="ExternalInput")     # b1 cols
    w2l = nc.dram_tensor("w2l", [NLOC, K, H], bf16, kind="ExternalInput")
    b2l = nc.dram_tensor("b2l", [NLOC, H], bf16, kind="ExternalInput")
    woutl = nc.dram_tensor("woutl", [H, VLOC], bf16, kind="ExternalInput")
    boutc = nc.dram_tensor("boutc", [P, VLOC // P], fp32, kind="ExternalInput")
    out = nc.dram_tensor("out", [VLOC, T], fp32, kind="ExternalOutput")

    rg = [list(range(CORES))]

    with tile.TileContext(nc) as tc:
        with (
            tc.tile_pool(name="const", bufs=1) as cpool,
            tc.tile_pool(name="dram", bufs=1, space="DRAM") as dpool,
        ):
            # ---- resident SBUF tensors ----
            w1sb = [cpool.tile([P, K], bf16, name=f"w1sb{i}") for i in range(NCH)]
            xsb = [cpool.tile([P, T], bf16, name=f"xsb{i}") for i in range(KT)]
            b1sb = cpool.tile([P, KT], fp32, name="b1sb")
            b2sb = cpool.tile([NLOC, H], bf16, name="b2sb")
            rlsb = cpool.tile([NLOC, T], bf16, name="rlsb")
            wosb = [cpool.tile([P, VLOC], bf16, name=f"wosb{i}") for i in range(HT)]
            bosb = cpool.tile([P, VLOC // P], fp32, name="bosb")
            # reduced r_ol^T accumulators (bf16), one per h-tile, full T
            rredsb = [cpool.tile([P, T], bf16, name=f"rredsb{i}") for i in range(HT)]

            # per-segment collective bounce buffers
            ar_in = [dpool.tile([H, TTH], bf16, name=f"ar_in{s}")
                     for s in range(NTH * NSEG)]
            ar_out = [dpool.tile([H, TTH], bf16, name=f"ar_out{s}",
                                 addr_space="Shared")
                      for s in range(NTH * NSEG)]

            # fc1's th0 critical chain first: rsb th0 + w1sb + b1
            nc.scalar.dma_start(bosb[:], boutc[:])
            for i in range(HT):
                nc.scalar.dma_start(wosb[i][:], woutl[i * P:(i + 1) * P, :])

            # ---- phase 1: fc1  x^T[k,t] = relu(sum_n W1T[n,k] r^T[n,t] + b1[k]) ----
            with (
                tc.tile_pool(name="xps", bufs=1, space="PSUM") as xpsp,
                tc.tile_pool(name="rsbp", bufs=1) as rsbp,
            ):
                rsb = [[rsbp.tile([P, TTH], bf16, name=f"rsb{i}_{th}")
                        for th in range(NTH)] for i in range(NCH)]
                for i in range(NCH):
                    nc.sync.dma_start(
                        rsb[i][0][:], rT[i * P:(i + 1) * P, 0:TTH]
                    )
                    nc.sync.dma_start(w1sb[i][:], w1t[i * P:(i + 1) * P, :])
                nc.sync.dma_start(b1sb[:], b1c[:])
                for i in range(NCH):
                    nc.sync.dma_start(
                        rsb[i][1][:], rT[i * P:(i + 1) * P, TTH:T]
                    )
                nc.sync.dma_start(b2sb[:], b2l[:])
                nc.sync.dma_start(rlsb[:], rloc[:])
                xps = [xpsp.tile([P, T], fp32, name=f"xps{i}") for i in range(KT)]
                for th in range(NTH):
                    for kt in range(KT):
                        for nch in range(NCH):
                            nc.tensor.matmul(
                                xps[kt][:, th * TTH:(th + 1) * TTH],
                                w1sb[nch][:, kt * P:(kt + 1) * P],
                                rsb[nch][th][:],
                                start=(nch == 0),
                                stop=(nch == NCH - 1),
                            )
                    for kt in range(KT):
                        nc.scalar.activation(
                            xsb[kt][:, th * TTH:(th + 1) * TTH],
                            xps[kt][:, th * TTH:(th + 1) * TTH], AF.Relu,
                            bias=b1sb[:, kt:kt + 1], scale=1.0,
                        )

            # ---- phase 2: main contraction, 4 pipelined segments ----
            with (
                tc.tile_pool(name="rolps", bufs=2, space="PSUM") as rolpsp,
                tc.tile_pool(name="w2p", bufs=3) as w2pool,
                tc.tile_pool(name="bcp", bufs=3) as bcpool,
                tc.tile_pool(name="xnp", bufs=2) as xnpool,
                tc.tile_pool(name="rolsbp", bufs=2) as rolsbp,
                tc.tile_pool(name="redp", bufs=2) as redp,
            ):
                cc_insts = []
                for th in range(NTH):
                    tsl = slice(th * TTH, (th + 1) * TTH)
                    for seg in range(NSEG):
                        sid = th * NSEG + seg
                        rolps = [rolpsp.tile([P, TTH], fp32, name=f"rolps{ht}",
                                             tag=f"rolps{ht}")
                                 for ht in range(HT)]
                        for j in range(NSEGSZ):
                            n = seg * NSEGSZ + j
                            bc = bcpool.tile([P, TTH], bf16, name="bc", tag="bc")
                            nc.gpsimd.dma_start(
                                bc[:], rloc[n:n + 1, tsl].to_broadcast((P, TTH))
                            )
                            w2t = []
                            for kc in range(KT):
                                w = w2pool.tile([P, H], bf16, name=f"w2t{kc}",
                                                tag=f"w2{kc}")
                                nc.sync.dma_start(w[:], w2l[n, kc * P:(kc + 1) * P, :])
                                w2t.append(w)
                            xn = []
                            for kc in range(KT):
                                x = xnpool.tile([P, TTH], bf16, name=f"xn{kc}",
                                                tag=f"xn{kc}")
                                nc.vector.tensor_mul(
                                    x[:], xsb[kc][:, tsl], bc[:]
                                )
                                xn.append(x)
                            first = j == 0
                            last = (seg != 0) and j == NSEGSZ - 1
                            for ht in range(HT):
                                for kc in range(KT):
                                    nc.tensor.matmul(
                                        rolps[ht][:],
                                        w2t[kc][:, ht * P:(ht + 1) * P],
                                        xn[kc][:],
                                        start=(first and kc == 0),
                                        stop=(last and kc == KT - 1),
                                    )
                        if seg == 0:
                            # b2 term closes seg 0's accumulation group (kept
                            # off the startup critical path)
                            for ht in range(HT):
                                nc.tensor.matmul(
                                    rolps[ht][:],
                                    b2sb[:, ht * P:(ht + 1) * P],
                                    rlsb[:, tsl],
                                    start=False,
                                    stop=True,
                                )
                        # segment readout -> AR (overlaps the next segment)
                        rolsb = [rolsbp.tile([P, TTH], bf16, name=f"rolsb{ht}",
                                             tag=f"rolsb{ht}")
                                 for ht in range(HT)]
                        for ht in range(HT):
                            nc.vector.tensor_copy(rolsb[ht][:], rolps[ht][:])
                            nc.sync.dma_start(
                                ar_in[sid][ht * P:(ht + 1) * P, :], rolsb[ht][:]
                            )
                        cc_insts.append(nc.gpsimd.collective_compute(
                            "AllReduce",
                            mybir.AluOpType.add,
                            replica_groups=rg,
                            ins=[ar_in[sid].opt()],
                            outs=[ar_out[sid].opt()],
                        ))

                # AR-output consumption is deferred here (not per segment):
                # any DMA that waits on a collective placed mid-stream drags
                # the shared queue semaphore thresholds with it and stalls the
                # W2 stream, starving the PE.  ar_out persists in DRAM, and
                # only the output projection needs it.
                # Ordering edge to the LAST collective keeps these loads (and
                # their shared-semaphore increments) out of the main-loop DMA
                # credit thresholds; the PE is saturated until then anyway.
                for th in range(NTH):
                    tsl = slice(th * TTH, (th + 1) * TTH)
                    for seg in range(NSEG):
                        sid = th * NSEG + seg
                        if seg == 0:
                            for ht in range(HT):
                                ld = nc.scalar.dma_start(
                                    rredsb[ht][:, tsl],
                                    ar_out[sid][ht * P:(ht + 1) * P, :],
                                )
                                add_dep_helper(ld.ins, cc_insts[-1].ins, False,
                                               "defer post-AR load")
                        else:
                            for ht in range(HT):
                                tmp = redp.tile([P, TTH], bf16, name="redtmp",
                                                tag="redtmp")
                                ld = nc.scalar.dma_start(
                                    tmp[:], ar_out[sid][ht * P:(ht + 1) * P, :]
                                )
                                add_dep_helper(ld.ins, cc_insts[-1].ins, False,
                                               "defer post-AR load")
                                nc.vector.tensor_add(
                                    rredsb[ht][:, tsl], rredsb[ht][:, tsl], tmp[:]
                                )

            # ---- phase 3: output projection (vocab shard), per token half ----
            with (
                tc.tile_pool(name="lgps", bufs=4, space="PSUM") as lgpsp,
                tc.tile_pool(name="osp", bufs=4) as opool,
            ):
                for th in range(NTH):
                    tsl = slice(th * TTH, (th + 1) * TTH)
                    for vt in range(VLOC // P):
                        lg = lgpsp.tile([P, TTH], fp32, name="lg", tag="lg")
                        for hc in range(HT):
                            nc.tensor.matmul(
                                lg[:],
                                wosb[hc][:, vt * P:(vt + 1) * P],
                                rredsb[hc][:, tsl],
                                start=(hc == 0),
                                stop=(hc == HT - 1),
                            )
                        ot = opool.tile([P, TTH], fp32, name="ot", tag="ot")
                        nc.scalar.activation(
                            ot[:], lg[:], AF.Identity,
                            bias=bosb[:, vt:vt + 1], scale=1.0,
                        )
                        nc.sync.dma_start(
                            out[vt * P:(vt + 1) * P, tsl], ot[:]
                        )

    nc.compile()
    return nc


def _get_nc():
    global _BUILT
    if _BUILT is None:
        _BUILT = _build()
    return _BUILT


def _prep_inputs(r_l, W1, b1, W2, b2, W_out, b_out):
    """Host-side sharding / layout only (transpose, slice, pad, dtype cast)."""
    r = np.ascontiguousarray(r_l.reshape(T, N))
    rT_b = np.ascontiguousarray(r.T).astype(BF16)                    # (N, T)
    w1t_b = np.ascontiguousarray(W1.T).astype(BF16)                  # (N, K)
    b1c = np.ascontiguousarray(b1.reshape(KT, P).T).astype(np.float32)
    W2b = W2.astype(BF16).reshape(H, N, K)                           # (h, n, k)
    b2r = b2.reshape(H, N)                                           # (h, n)
    wout_pad = np.zeros((VPAD, H), dtype=BF16)
    wout_pad[:VOCAB] = W_out.astype(BF16)
    bout_pad = np.zeros((VPAD,), dtype=np.float32)
    bout_pad[:VOCAB] = b_out

    in_maps = []
    for c in range(CORES):
        nsl = slice(c * NLOC, (c + 1) * NLOC)
        vsl = slice(c * VLOC, (c + 1) * VLOC)
        in_maps.append({
            "rT": rT_b,
            "rloc": np.ascontiguousarray(rT_b[nsl]),
            "w1t": w1t_b,
            "b1c": b1c,
            "w2l": np.ascontiguousarray(W2b[:, nsl, :].transpose(1, 2, 0)),
            "b2l": np.ascontiguousarray(b2r[:, nsl].T).astype(BF16),
            "woutl": np.ascontiguousarray(wout_pad[vsl].T),
            "boutc": np.ascontiguousarray(bout_pad[vsl].reshape(VLOC // P, P).T),
        })
    return in_maps


def _run(inputs, trace=False, tmpdir=None):
    from concourse import bass_utils

    nc = _get_nc()
    in_maps = _prep_inputs(**inputs)
    res = bass_utils.run_bass_kernel_spmd(
        nc, in_maps, core_ids=list(range(CORES)), trace=trace, tmpdir=tmpdir,
    )
    parts = [res.results[c]["out"] for c in range(CORES)]
    full = np.concatenate(parts, axis=0)          # (VPAD, T)
    logits = np.ascontiguousarray(full[:VOCAB].T)  # (T, VOCAB)
    out = logits.reshape(4, 256, VOCAB).astype(np.float32)
    return out, res


def kernel(**inputs):
    out, _ = _run(inputs, trace=False)
    return out
